# revision 7
# baseline (speedup 1.0000x reference)
"""Trainium2 Bass kernel for nn_LocalTokenDispatcher (MoE routing).

Mathematical collapse: the reference does histogram -> stable sort by
expert -> gather x rows -> scale by sorted scores -> scatter-add back to
token order, with an *identity* expert in the middle. The scatter-add
sums, for each token t, exactly the TOP_K slots that originated from t,
so the whole pipeline reduces to

    out[t, :]  = x[t, :] * s[t,0] + x[t, :] * s[t,1]
    counts[e]  = #{(t,k) : selected_experts_indices[t,k] == e}

which is a memory-bound row-scale over 256 MB plus a 64-bin histogram.

Sharding: tokens split 8 ways (2048 tokens/core, data parallel). Each
core streams its 32 MB x-shard through SBUF, multiplies by the
per-token score sum on the Vector engine, writes back, and computes a
local histogram (one-hot via is_equal against an iota row, then a
partition-reduction matmul with a ones vector — exact for these small
integer values). Host concatenates out-shards and sums the 8 local
histograms (exact in fp32: counts <= 32768 < 2^24).

Token->tile mapping inside a core: token t = p*16 + g sits on SBUF
partition p of column-tile g. With that mapping the per-token score sums
(computed once into a (128,16) tile) are consumed tile-by-tile as plain
column slices — no cross-partition traffic anywhere.
"""

import numpy as np

import concourse.bass as bass
import concourse.tile as tile
from concourse import bacc, mybir
from concourse.bass_utils import run_bass_kernel_spmd

NUM_EXPERTS = 64
TOP_K = 2
NUM_TOKENS = 16384
DIM = 4096
N_CORES = 8
TPC = NUM_TOKENS // N_CORES          # 2048 tokens per core
G = TPC // 128                       # 16 column-tiles per core
JSLOTS = TPC * TOP_K // 128          # 32 expert-id slots per partition

_STATE = {}


def _build_nc():
    f32 = mybir.dt.float32
    i32 = mybir.dt.int32

    nc = bacc.Bacc(None)
    x_d = nc.declare_dram_parameter("x", [TPC, DIM], f32, isOutput=False)
    ts_d = nc.declare_dram_parameter("ts", [TPC, TOP_K], f32, isOutput=False)
    sei_d = nc.declare_dram_parameter("sei", [TPC, TOP_K], i32, isOutput=False)
    out_d = nc.declare_dram_parameter("out", [TPC, DIM], f32, isOutput=True)
    cnt_d = nc.declare_dram_parameter("cnt", [1, NUM_EXPERTS], f32, isOutput=True)

    # DRAM views: token t = p*G + g  ->  [partition p, tile g, :]
    x_v = x_d[:].rearrange("(p g) d -> p g d", g=G)
    out_v = out_d[:].rearrange("(p g) d -> p g d", g=G)

    with tile.TileContext(nc) as tc:
        with (
            tc.tile_pool(name="singles", bufs=1) as singles,
            tc.tile_pool(name="xpool", bufs=6) as xpool,
        ):
            # ---- per-token score sums ------------------------------------
            st = singles.tile([128, G * TOP_K], f32)           # scores, flat
            nc.sync.dma_start(st[:], ts_d[:].rearrange("(p w) k -> p (w k)", p=128))
            st_v = st[:].rearrange("p (i k) -> p i k", k=TOP_K)
            sums16 = singles.tile([128, G], f32)               # sums16[p, g] = scale of token p*G+g
            nc.vector.tensor_add(sums16[:], st_v[:, :, 0], st_v[:, :, 1])

            # ---- histogram ----------------------------------------------
            svt = singles.tile([128, JSLOTS], i32)             # expert ids, flat
            nc.sync.dma_start(svt[:], sei_d[:].rearrange("(p w) k -> p (w k)", p=128))
            svt_f = singles.tile([128, JSLOTS], f32)           # ids as f32 (exact, < 64)
            nc.gpsimd.tensor_copy(svt_f[:], svt[:])
            iota_t = singles.tile([128, NUM_EXPERTS], f32)     # each row: 0..63
            nc.gpsimd.iota(
                iota_t[:], pattern=[[1, NUM_EXPERTS]], base=0, channel_multiplier=0,
                allow_small_or_imprecise_dtypes=True,
            )

            eq = singles.tile([128, JSLOTS, NUM_EXPERTS], f32)  # one-hot per slot
            for j in range(JSLOTS):
                nc.gpsimd.tensor_scalar(
                    eq[:, j, :], iota_t[:], svt_f[:, j : j + 1], None,
                    op0=mybir.AluOpType.is_equal,
                )
            acc = singles.tile([128, NUM_EXPERTS], f32)        # per-partition histogram
            nc.vector.tensor_reduce(
                acc[:], eq[:].rearrange("p j e -> p e j"),
                axis=mybir.AxisListType.X, op=mybir.AluOpType.add,
            )
            cnt_row = singles.tile([1, NUM_EXPERTS], f32)      # counts[e] = sum_p acc[p, e]
            nc.gpsimd.tensor_reduce(
                cnt_row[:], acc[:], axis=mybir.AxisListType.C, op=mybir.AluOpType.add,
            )
            nc.sync.dma_start(cnt_d[:], cnt_row[:])

            # ---- the memory-bound row scale ------------------------------
            for g in range(G):
                xt = xpool.tile([128, DIM], f32)
                nc.gpsimd.dma_start(xt[:], x_v[:, g, :])
                nc.scalar.mul(xt[:], xt[:], sums16[:, g : g + 1])
                nc.gpsimd.dma_start(out_v[:, g, :], xt[:])

    nc.finalize()
    return nc


def _get_nc():
    if "nc" not in _STATE:
        _STATE["nc"] = _build_nc()
    return _STATE["nc"]


def _make_in_maps(x, top_scores, selected_experts_indices):
    x = np.ascontiguousarray(np.asarray(x, dtype=np.float32))
    ts = np.ascontiguousarray(np.asarray(top_scores, dtype=np.float32))
    sei = np.ascontiguousarray(np.asarray(selected_experts_indices).astype(np.int32))
    in_maps = []
    for i in range(N_CORES):
        sl = slice(i * TPC, (i + 1) * TPC)
        in_maps.append({"x": x[sl], "ts": ts[sl], "sei": sei[sl]})
    return in_maps


def _run(x, top_scores, selected_experts_indices, trace=False):
    nc = _get_nc()
    in_maps = _make_in_maps(x, top_scores, selected_experts_indices)
    res = run_bass_kernel_spmd(nc, in_maps, core_ids=list(range(N_CORES)), trace=trace)
    out = np.concatenate([res.results[i]["out"] for i in range(N_CORES)], axis=0)
    counts = np.zeros(NUM_EXPERTS, dtype=np.float32)
    for i in range(N_CORES):
        counts += res.results[i]["cnt"].reshape(NUM_EXPERTS)
    return (out, counts), res


def kernel(x, top_scores, selected_experts_indices, num_tokens_per_expert=None):
    (out, counts), _ = _run(x, top_scores, selected_experts_indices)
    return out, counts


# revision 9
# speedup vs baseline: 1.2065x; 1.2065x over previous
"""Trainium2 Bass kernel for nn_LocalTokenDispatcher (MoE routing).

Mathematical collapse: the reference does histogram -> stable sort by
expert -> gather x rows -> scale by sorted scores -> scatter-add back to
token order, with an *identity* expert in the middle. The scatter-add
sums, for each token t, exactly the TOP_K slots that originated from t,
so the whole pipeline reduces to

    out[t, :]  = x[t, :] * s[t,0] + x[t, :] * s[t,1]
    counts[e]  = #{(t,k) : selected_experts_indices[t,k] == e}

which is a memory-bound row-scale over 256 MB plus a 64-bin histogram.

Sharding: tokens split 8 ways (2048 tokens/core, data parallel). Each
core streams its 32 MB x-shard through SBUF, multiplies by the
per-token score sum on the Vector engine, writes back, and computes a
local histogram (one-hot via is_equal against an iota row, then a
partition-reduction matmul with a ones vector — exact for these small
integer values). Host concatenates out-shards and sums the 8 local
histograms (exact in fp32: counts <= 32768 < 2^24).

Token->tile mapping inside a core: token t = p*16 + g sits on SBUF
partition p of column-tile g. With that mapping the per-token score sums
(computed once into a (128,16) tile) are consumed tile-by-tile as plain
column slices — no cross-partition traffic anywhere.
"""

import numpy as np

import concourse.bass as bass
import concourse.tile as tile
from concourse import bacc, mybir
from concourse.bass_utils import run_bass_kernel_spmd

NUM_EXPERTS = 64
TOP_K = 2
NUM_TOKENS = 16384
DIM = 4096
N_CORES = 8
TPC = NUM_TOKENS // N_CORES          # 2048 tokens per core
G = TPC // 128                       # 16 column-tiles per core
JSLOTS = TPC * TOP_K // 128          # 32 expert-id slots per partition

_STATE = {}


def _build_nc():
    f32 = mybir.dt.float32
    i32 = mybir.dt.int32

    nc = bacc.Bacc(None)
    x_d = nc.declare_dram_parameter("x", [TPC, DIM], f32, isOutput=False)
    ts_d = nc.declare_dram_parameter("ts", [TPC, TOP_K], f32, isOutput=False)
    sei_d = nc.declare_dram_parameter("sei", [TPC, TOP_K], i32, isOutput=False)
    out_d = nc.declare_dram_parameter("out", [TPC, DIM], f32, isOutput=True)
    cnt_d = nc.declare_dram_parameter("cnt", [NUM_EXPERTS, 1], f32, isOutput=True)

    # DRAM views: token t = p*G + g  ->  [partition p, tile g, :]
    x_v = x_d[:].rearrange("(p g) d -> p g d", g=G)
    out_v = out_d[:].rearrange("(p g) d -> p g d", g=G)

    with tile.TileContext(nc) as tc:
        with (
            tc.tile_pool(name="singles", bufs=1) as singles,
            tc.tile_pool(name="xpool", bufs=6) as xpool,
            tc.tile_pool(name="psum", bufs=1, space="PSUM") as psum,
        ):
            # iota is GpSimd-only; issue it first so the Q7 engine spends the
            # rest of the kernel exclusively on SWDGE DMA descriptor-gen.
            iota_t = singles.tile([128, NUM_EXPERTS], f32)     # each row: 0..63
            nc.gpsimd.iota(
                iota_t[:], pattern=[[1, NUM_EXPERTS]], base=0, channel_multiplier=0,
                allow_small_or_imprecise_dtypes=True,
            )

            # ---- per-token score sums ------------------------------------
            st = singles.tile([128, G * TOP_K], f32)           # scores, flat
            nc.sync.dma_start(st[:], ts_d[:].rearrange("(p w) k -> p (w k)", p=128))
            st_v = st[:].rearrange("p (i k) -> p i k", k=TOP_K)
            sums16 = singles.tile([128, G], f32)               # sums16[p, g] = scale of token p*G+g
            nc.vector.tensor_add(sums16[:], st_v[:, :, 0], st_v[:, :, 1])

            # ---- the memory-bound row scale ------------------------------
            for g in range(G):
                xt = xpool.tile([128, DIM], f32)
                nc.gpsimd.dma_start(xt[:], x_v[:, g, :])
                nc.vector.tensor_scalar(
                    xt[:], xt[:], sums16[:, g : g + 1], None,
                    op0=mybir.AluOpType.mult,
                )
                nc.gpsimd.dma_start(out_v[:, g, :], xt[:])

            # ---- histogram (DVE one-hot + PE partition-sum; Q7 untouched) --
            svt = singles.tile([128, JSLOTS], i32)             # expert ids, flat
            nc.sync.dma_start(svt[:], sei_d[:].rearrange("(p w) k -> p (w k)", p=128))
            svt_f = singles.tile([128, JSLOTS], f32)           # ids as f32 (exact, < 64)
            nc.vector.tensor_copy(svt_f[:], svt[:])

            eq = singles.tile([128, JSLOTS, NUM_EXPERTS], f32)  # one-hot per slot
            for j in range(JSLOTS):
                nc.vector.tensor_scalar(
                    eq[:, j, :], iota_t[:], svt_f[:, j : j + 1], None,
                    op0=mybir.AluOpType.is_equal,
                )
            acc = singles.tile([128, NUM_EXPERTS], f32)        # per-partition histogram
            nc.vector.tensor_reduce(
                acc[:], eq[:].rearrange("p j e -> p e j"),
                axis=mybir.AxisListType.X, op=mybir.AluOpType.add,
            )
            ones = singles.tile([128, 1], f32)
            nc.vector.memset(ones[:], 1.0)
            cnt_ps = psum.tile([NUM_EXPERTS, 1], f32)          # counts[e] = sum_p acc[p, e]
            nc.tensor.matmul(cnt_ps[:], acc[:], ones[:], start=True, stop=True)
            cnt_sb = singles.tile([NUM_EXPERTS, 1], f32)
            nc.vector.tensor_copy(cnt_sb[:], cnt_ps[:])
            nc.sync.dma_start(cnt_d[:], cnt_sb[:])

    nc.finalize()
    return nc


def _get_nc():
    if "nc" not in _STATE:
        _STATE["nc"] = _build_nc()
    return _STATE["nc"]


def _make_in_maps(x, top_scores, selected_experts_indices):
    x = np.ascontiguousarray(np.asarray(x, dtype=np.float32))
    ts = np.ascontiguousarray(np.asarray(top_scores, dtype=np.float32))
    sei = np.ascontiguousarray(np.asarray(selected_experts_indices).astype(np.int32))
    in_maps = []
    for i in range(N_CORES):
        sl = slice(i * TPC, (i + 1) * TPC)
        in_maps.append({"x": x[sl], "ts": ts[sl], "sei": sei[sl]})
    return in_maps


def _run(x, top_scores, selected_experts_indices, trace=False):
    nc = _get_nc()
    in_maps = _make_in_maps(x, top_scores, selected_experts_indices)
    res = run_bass_kernel_spmd(nc, in_maps, core_ids=list(range(N_CORES)), trace=trace)
    out = np.concatenate([res.results[i]["out"] for i in range(N_CORES)], axis=0)
    counts = np.zeros(NUM_EXPERTS, dtype=np.float32)
    for i in range(N_CORES):
        counts += res.results[i]["cnt"].reshape(NUM_EXPERTS)
    return (out, counts), res


def kernel(x, top_scores, selected_experts_indices, num_tokens_per_expert=None):
    (out, counts), _ = _run(x, top_scores, selected_experts_indices)
    return out, counts


# revision 11
# speedup vs baseline: 1.2409x; 1.0285x over previous
"""Trainium2 Bass kernel for nn_LocalTokenDispatcher (MoE routing).

Mathematical collapse: the reference does histogram -> stable sort by
expert -> gather x rows -> scale by sorted scores -> scatter-add back to
token order, with an *identity* expert in the middle. The scatter-add
sums, for each token t, exactly the TOP_K slots that originated from t,
so the whole pipeline reduces to

    out[t, :]  = x[t, :] * s[t,0] + x[t, :] * s[t,1]
    counts[e]  = #{(t,k) : selected_experts_indices[t,k] == e}

which is a memory-bound row-scale over 256 MB plus a 64-bin histogram.

Sharding: tokens split 8 ways (2048 tokens/core, data parallel). Each
core streams its 32 MB x-shard through SBUF, multiplies by the
per-token score sum on the Vector engine, writes back, and computes a
local histogram (one-hot via is_equal against an iota row, then a
partition-reduction matmul with a ones vector — exact for these small
integer values). Host concatenates out-shards and sums the 8 local
histograms (exact in fp32: counts <= 32768 < 2^24).

Token->tile mapping inside a core: token t = p*16 + g sits on SBUF
partition p of column-tile g. With that mapping the per-token score sums
(computed once into a (128,16) tile) are consumed tile-by-tile as plain
column slices — no cross-partition traffic anywhere.
"""

import numpy as np

import concourse.bass as bass
import concourse.tile as tile
from concourse import bacc, mybir
from concourse.bass_utils import run_bass_kernel_spmd

NUM_EXPERTS = 64
TOP_K = 2
NUM_TOKENS = 16384
DIM = 4096
N_CORES = 8
TPC = NUM_TOKENS // N_CORES          # 2048 tokens per core
G = TPC // 128                       # 16 column-tiles per core
JSLOTS = TPC * TOP_K // 128          # 32 expert-id slots per partition

# -- tuning knobs (set before first kernel() call) --
TILES_PER_DMA = 2                    # column-tiles per x DMA (2 -> 4MB transfers)
XPOOL_BUFS = 4
STORE_ENGINE = lambda nc: nc.sync    # engine issuing output DMAs

_STATE = {}


def _build_nc():
    f32 = mybir.dt.float32
    i32 = mybir.dt.int32

    nc = bacc.Bacc(None)
    x_d = nc.declare_dram_parameter("x", [TPC, DIM], f32, isOutput=False)
    ts_d = nc.declare_dram_parameter("ts", [TPC, TOP_K], f32, isOutput=False)
    sei_d = nc.declare_dram_parameter("sei", [TPC, TOP_K], i32, isOutput=False)
    out_d = nc.declare_dram_parameter("out", [TPC, DIM], f32, isOutput=True)
    cnt_d = nc.declare_dram_parameter("cnt", [NUM_EXPERTS, 1], f32, isOutput=True)

    # DRAM views: token t = p*G + g  ->  [partition p, tile g, :]
    x_v = x_d[:].rearrange("(p g) d -> p g d", g=G)
    out_v = out_d[:].rearrange("(p g) d -> p g d", g=G)

    with tile.TileContext(nc) as tc:
        with (
            tc.tile_pool(name="singles", bufs=1) as singles,
            tc.tile_pool(name="xpool", bufs=XPOOL_BUFS) as xpool,
            tc.tile_pool(name="psum", bufs=1, space="PSUM") as psum,
        ):
            # iota is GpSimd-only; issue it first so the Q7 engine spends the
            # rest of the kernel exclusively on SWDGE DMA descriptor-gen.
            iota_t = singles.tile([128, NUM_EXPERTS], f32)     # each row: 0..63
            nc.gpsimd.iota(
                iota_t[:], pattern=[[1, NUM_EXPERTS]], base=0, channel_multiplier=0,
                allow_small_or_imprecise_dtypes=True,
            )

            # ---- per-token score sums ------------------------------------
            st = singles.tile([128, G * TOP_K], f32)           # scores, flat
            nc.sync.dma_start(st[:], ts_d[:].rearrange("(p w) k -> p (w k)", p=128))
            st_v = st[:].rearrange("p (i k) -> p i k", k=TOP_K)
            sums16 = singles.tile([128, G], f32)               # sums16[p, g] = scale of token p*G+g
            nc.vector.tensor_add(sums16[:], st_v[:, :, 0], st_v[:, :, 1])

            # ---- the memory-bound row scale ------------------------------
            # TILES_PER_DMA column-tiles per transfer; loads ride the 8
            # SWDGE queues (Q7 descgen never blocks on compute), stores go
            # out the sync-engine HWDGE ring.
            for gg in range(G // TILES_PER_DMA):
                g0 = gg * TILES_PER_DMA
                xt = xpool.tile([128, TILES_PER_DMA, DIM], f32)
                nc.gpsimd.dma_start(xt[:], x_v[:, g0 : g0 + TILES_PER_DMA, :])
                for u in range(TILES_PER_DMA):
                    g = g0 + u
                    nc.vector.tensor_scalar(
                        xt[:, u, :], xt[:, u, :], sums16[:, g : g + 1], None,
                        op0=mybir.AluOpType.mult,
                    )
                STORE_ENGINE(nc).dma_start(out_v[:, g0 : g0 + TILES_PER_DMA, :], xt[:])

            # ---- histogram (DVE one-hot + PE partition-sum; Q7 untouched) --
            svt = singles.tile([128, JSLOTS], i32)             # expert ids, flat
            nc.sync.dma_start(svt[:], sei_d[:].rearrange("(p w) k -> p (w k)", p=128))
            svt_f = singles.tile([128, JSLOTS], f32)           # ids as f32 (exact, < 64)
            nc.vector.tensor_copy(svt_f[:], svt[:])

            eq = singles.tile([128, JSLOTS, NUM_EXPERTS], f32)  # one-hot per slot
            for j in range(JSLOTS):
                nc.vector.tensor_scalar(
                    eq[:, j, :], iota_t[:], svt_f[:, j : j + 1], None,
                    op0=mybir.AluOpType.is_equal,
                )
            acc = singles.tile([128, NUM_EXPERTS], f32)        # per-partition histogram
            nc.vector.tensor_reduce(
                acc[:], eq[:].rearrange("p j e -> p e j"),
                axis=mybir.AxisListType.X, op=mybir.AluOpType.add,
            )
            ones = singles.tile([128, 1], f32)
            nc.vector.memset(ones[:], 1.0)
            cnt_ps = psum.tile([NUM_EXPERTS, 1], f32)          # counts[e] = sum_p acc[p, e]
            nc.tensor.matmul(cnt_ps[:], acc[:], ones[:], start=True, stop=True)
            cnt_sb = singles.tile([NUM_EXPERTS, 1], f32)
            nc.vector.tensor_copy(cnt_sb[:], cnt_ps[:])
            nc.sync.dma_start(cnt_d[:], cnt_sb[:])

    nc.finalize()
    return nc


def _get_nc():
    if "nc" not in _STATE:
        _STATE["nc"] = _build_nc()
    return _STATE["nc"]


def _make_in_maps(x, top_scores, selected_experts_indices):
    x = np.ascontiguousarray(np.asarray(x, dtype=np.float32))
    ts = np.ascontiguousarray(np.asarray(top_scores, dtype=np.float32))
    sei = np.ascontiguousarray(np.asarray(selected_experts_indices).astype(np.int32))
    in_maps = []
    for i in range(N_CORES):
        sl = slice(i * TPC, (i + 1) * TPC)
        in_maps.append({"x": x[sl], "ts": ts[sl], "sei": sei[sl]})
    return in_maps


def _run(x, top_scores, selected_experts_indices, trace=False):
    nc = _get_nc()
    in_maps = _make_in_maps(x, top_scores, selected_experts_indices)
    res = run_bass_kernel_spmd(nc, in_maps, core_ids=list(range(N_CORES)), trace=trace)
    out = np.concatenate([res.results[i]["out"] for i in range(N_CORES)], axis=0)
    counts = np.zeros(NUM_EXPERTS, dtype=np.float32)
    for i in range(N_CORES):
        counts += res.results[i]["cnt"].reshape(NUM_EXPERTS)
    return (out, counts), res


def kernel(x, top_scores, selected_experts_indices, num_tokens_per_expert=None):
    (out, counts), _ = _run(x, top_scores, selected_experts_indices)
    return out, counts
